# revision 1
# baseline (speedup 1.0000x reference)
# Linear-chain CRF log-marginals on 8 Trainium2 NeuronCores.
#
# alpha/beta recurrences are run in the exp domain: the per-step
# LSE_k(alpha[k] + T[k,j]) becomes a matvec u @ exp(T) on the PE array
# (fp16 operands, fp32 PSUM accumulate), with a constant per-step prescale
# exp(-MU) folded into exp(scores) and a periodic data-dependent renorm to
# keep the fp16 carry in range.  The sequence is split into many short
# chunks run speculatively in lockstep (the chain mixes in ~10 steps, so a
# W-step warmup makes each chunk's carry exact up to a common constant);
# 32 chunk-scans per core share each stationary-weight load.  Chunk
# constants are resolved on the host in fp64 by matching one overlap row
# per boundary.  A second tiny device pass computes
# log(Vf*Vb) + scores + rho for the final marginals.
import numpy as np
from contextlib import ExitStack

import concourse.bass as bass
import concourse.tile as tile
from concourse import bacc, mybir
from concourse.bass_utils import run_bass_kernel_spmd

F16 = mybir.dt.float16
F32 = mybir.dt.float32
AFT = mybir.ActivationFunctionType

# ---- problem constants ----
N, C = 8192, 1024
P = 128                  # partitions
CT = C // P              # 8 contraction/output tiles of 128 classes
NCORE = 8

# ---- algorithm parameters ----
NS = 32                  # lockstep scans per core
NCH = 4 * NS             # chunks per direction (4 cores each direction)
L = N // NCH             # 64 positions per chunk
W = 32                   # warmup steps per speculative chunk
R = W + L + 1            # rows per scan: init row + L+W steps
MU = 7.927               # constant per-step log-prescale
B0 = 4.0                 # init offset: u_0 = exp(s_0 - B0)
RN = 32                  # renorm cadence (sigma measured at m, applied at m+1)
BCS = 64.0               # renorm rescale target (sum -> 64)
G = 8                    # steps per DMA/exp group
NSC = CT * NS            # 256 carry columns per core
RENORM_STEPS = list(range(RN, R - 1, RN))
NREN = max(1, len(RENORM_STEPS))

_scan_nc = None
_epi_nc = None
TIMINGS = {}


# ---------------------------------------------------------------- builders
def build_scan_nc(steps=None, repeats=1, timing_loop=None):
    nsteps = R if steps is None else steps
    nc = bacc.Bacc(None, target_bir_lowering=False)
    tmat = nc.declare_dram_parameter("tmat", [P, C * CT], F32, isOutput=False)
    srows = nc.declare_dram_parameter("srows", [P, R * NSC], F32, isOutput=False)
    vdump = nc.declare_dram_parameter("vdump", [P, R * NSC], F32, isOutput=True)
    fdump = nc.declare_dram_parameter("fdump", [1, NREN * NS], F16, isOutput=True)

    ngroups = (R + G - 1) // G

    with tile.TileContext(nc) as tc, ExitStack() as ctx:
        const = ctx.enter_context(tc.tile_pool(name="const", bufs=1))
        mpool = ctx.enter_context(tc.tile_pool(name="m16", bufs=1))
        tin = ctx.enter_context(tc.tile_pool(name="tin", bufs=2))
        spool = ctx.enter_context(tc.tile_pool(name="sin", bufs=2))
        espool = ctx.enter_context(tc.tile_pool(name="es", bufs=2))
        vpool = ctx.enter_context(tc.tile_pool(name="vst", bufs=2))
        upool = ctx.enter_context(tc.tile_pool(name="u", bufs=3))
        fpool = ctx.enter_context(tc.tile_pool(name="f", bufs=2))
        psA = ctx.enter_context(tc.tile_pool(name="psA", bufs=2, space="PSUM"))
        psB = ctx.enter_context(tc.tile_pool(name="psB", bufs=2, space="PSUM"))
        psS = ctx.enter_context(tc.tile_pool(name="psS", bufs=1, space="PSUM"))
        psC = ctx.enter_context(tc.tile_pool(name="psC", bufs=1, space="PSUM"))

        ones = const.tile([P, 1], F16)
        nc.any.memset(ones[:], 1.0)
        bc64 = const.tile([1, P], F16)
        nc.any.memset(bc64[:], BCS)
        biasmu = const.tile([P, 1], F32)
        nc.any.memset(biasmu[:], -MU)
        fexp = const.tile([1, NREN * NS], F16)
        nc.any.memset(fexp[:], 1.0)

        # M16 = exp(tmat) fp16, staged in halves through a fp32 temp
        m16 = mpool.tile([P, C * CT], F16)
        for h in range(2):
            tt = tin.tile([P, C * CT // 2], F32)
            nc.sync.dma_start(tt[:], tmat[:, h * 4096:(h + 1) * 4096])
            nc.scalar.activation(m16[:, h * 4096:(h + 1) * 4096], tt[:], AFT.Exp)

        # es groups: DMA srows slice -> exp(x - MU)
        es_tiles = [None] * ngroups

        def emit_group(g):
            lo = g * G * NSC
            hi = min(R, (g + 1) * G) * NSC
            st = spool.tile([P, G * NSC], F32)
            nc.sync.dma_start(st[:, 0:hi - lo], srows[:, lo:hi])
            et = espool.tile([P, G * NSC], F32)
            nc.scalar.activation(et[:, 0:hi - lo], st[:, 0:hi - lo], AFT.Exp,
                                 bias=biasmu[:])
            es_tiles[g] = et

        loop_cm = tc.For_i(0, timing_loop, 1) if timing_loop else ExitStack()
        with loop_cm:
            emit_group(0)

            # r = 0: V0 = es_row0 * e^(MU-B0); u0 = fp16(V0)
            vst = vpool.tile([P, G * NSC], F32)
            nc.scalar.mul(vst[:, 0:NSC], es_tiles[0][:, 0:NSC], float(np.exp(MU - B0)))
            u_prev = upool.tile([P, NSC], F16)
            nc.vector.tensor_copy(u_prev[:], vst[:, 0:NSC])

            fbc16 = None
            for r in range(1, nsteps):
                g, slot = divmod(r, G)
                if slot == 0:           # new group: fresh vst tile, prefetch es
                    if g + 1 < ngroups:
                        pass
                    if es_tiles[g] is None:
                        emit_group(g)
                    vst = vpool.tile([P, G * NSC], F32)
                if slot == 0 and g + 1 < ngroups and es_tiles[g + 1] is None:
                    emit_group(g + 1)
                es = es_tiles[g]
                off = slot * NSC

                psa = psA.tile([P, 4 * NS], F32)
                psb = psB.tile([P, 4 * NS], F32)
                for jt in range(CT):
                    tgt = psa if jt < 4 else psb
                    col = (jt % 4) * NS
                    for kt in range(CT):
                        nc.tensor.matmul(
                            tgt[:, col:col + NS],
                            m16[:, (kt * CT + jt) * P:(kt * CT + jt + 1) * P],
                            u_prev[:, kt * NS:(kt + 1) * NS],
                            start=(jt % 4 == 0 and kt == 0),
                            stop=(jt % 4 == 3 and kt == CT - 1),
                        )
                # V out (fp32) via ScalarE (fast PSUM reads)
                nc.scalar.copy(vst[:, off:off + 4 * NS], psa[:])
                nc.scalar.copy(vst[:, off + 4 * NS:off + 8 * NS], psb[:])
                # u_next = V * es  (fp16), halves so each TT waits only on ACT
                u_nxt = upool.tile([P, NSC], F16)
                nc.vector.tensor_mul(u_nxt[:, 0:4 * NS], vst[:, off:off + 4 * NS],
                                     es[:, off:off + 4 * NS])
                nc.vector.tensor_mul(u_nxt[:, 4 * NS:NSC],
                                     vst[:, off + 4 * NS:off + 8 * NS],
                                     es[:, off + 4 * NS:off + 8 * NS])
                if r - 1 in RENORM_STEPS:   # deferred renorm apply
                    nc.vector.tensor_mul(u_nxt[:], u_nxt[:], fbc16[:])
                if r in RENORM_STEPS:       # measure sigma on u_r
                    ridx = RENORM_STEPS.index(r)
                    sig = psS.tile([P, NS], F32)
                    for kt in range(CT):
                        nc.tensor.matmul(sig[0:1, :], ones[:],
                                         u_nxt[:, kt * NS:(kt + 1) * NS],
                                         start=(kt == 0), stop=(kt == CT - 1))
                    f32t = fpool.tile([1, NS], F32)
                    nc.vector.reciprocal(f32t[:], sig[0:1, :])
                    nc.vector.tensor_copy(fexp[0:1, ridx * NS:(ridx + 1) * NS],
                                          f32t[:])
                    f8 = fpool.tile([1, NSC], F16)
                    for kt in range(CT):
                        nc.vector.tensor_copy(f8[0:1, kt * NS:(kt + 1) * NS],
                                              fexp[0:1, ridx * NS:(ridx + 1) * NS])
                    pbc = psC.tile([P, NSC], F32)
                    nc.tensor.matmul(pbc[:], bc64[:], f8[:], start=True, stop=True)
                    fbc16 = fpool.tile([P, NSC], F16)
                    nc.vector.tensor_copy(fbc16[:], pbc[:])
                if slot == G - 1 or r == R - 1:
                    lo = g * G * NSC
                    hi = min(R, (g + 1) * G) * NSC
                    nc.sync.dma_start(vdump[:, lo:hi], vst[:, 0:hi - lo])
                u_prev = u_nxt


        nc.sync.dma_start(fdump[:], fexp[:])
    nc.finalize()
    return nc


def build_epi_nc():
    nc = bacc.Bacc(None, target_bir_lowering=False)
    TI = N // NCORE // P     # 8 position tiles per core
    vf = nc.declare_dram_parameter("vf", [P, TI * C], F32, isOutput=False)
    vb = nc.declare_dram_parameter("vb", [P, TI * C], F32, isOutput=False)
    sp = nc.declare_dram_parameter("sp", [P, TI * C], F32, isOutput=False)
    out = nc.declare_dram_parameter("out", [P, TI * C], F32, isOutput=True)

    with tile.TileContext(nc) as tc, ExitStack() as ctx:
        pool = ctx.enter_context(tc.tile_pool(name="sb", bufs=3))
        for ti in range(TI):
            sl = slice(ti * C, (ti + 1) * C)
            a = pool.tile([P, C], F32)
            nc.sync.dma_start(a[:], vf[:, sl])
            b = pool.tile([P, C], F32)
            nc.sync.dma_start(b[:], vb[:, sl])
            s = pool.tile([P, C], F32)
            nc.sync.dma_start(s[:], sp[:, sl])
            m = pool.tile([P, C], F32)
            nc.vector.tensor_mul(m[:], a[:], b[:])
            lg = pool.tile([P, C], F32)
            nc.scalar.activation(lg[:], m[:], AFT.Ln)
            o = pool.tile([P, C], F32)
            nc.vector.tensor_add(o[:], lg[:], s[:])
            nc.sync.dma_start(out[:, sl], o[:])
    nc.finalize()
    return nc


# ---------------------------------------------------------------- host prep
def build_chunk_scores(sdir):
    """Per-direction chunk score rows [NCH, R, C] (fp32, zero-padded)."""
    SS = np.zeros((NCH, R, C), np.float32)
    for g in range(NCH):
        lo = 0 if g == 0 else g * L - W
        seg = sdir[lo:min(lo + R, N)]
        SS[g, :seg.shape[0]] = seg
    return SS


def prep_scan_inputs(scores, T):
    maps = []
    for d, (Tp, sdir) in enumerate([(T, scores), (T.T, scores[::-1])]):
        tmat = np.ascontiguousarray(
            Tp.reshape(P, CT, P, CT).transpose(0, 1, 3, 2).reshape(P, CT * CT * P),
            dtype=np.float32)
        SS = build_chunk_scores(sdir)
        for cidx in range(4):
            SSc = SS[cidx * NS:(cidx + 1) * NS]          # [NS, R, C]
            srows = np.ascontiguousarray(
                SSc.reshape(NS, R, P, CT).transpose(2, 1, 3, 0).reshape(P, R * NSC))
            maps.append({"tmat": tmat, "srows": srows})
    return maps


def parse_scan_results(res):
    """-> per direction: Vg [NCH][R, C] fp32, logf64 [NCH][R] fp64."""
    out = []
    for d in range(2):
        Vg, logf = [], []
        for cidx in range(4):
            r = res[d * 4 + cidx]
            vd = r["vdump"].reshape(P, R, CT, NS)
            fd = r["fdump"].reshape(NREN, NS)
            for s in range(NS):
                Vg.append(np.ascontiguousarray(
                    vd[:, :, :, s].transpose(1, 0, 2).reshape(R, C)))
                lf = np.zeros(R)
                for mi, m in enumerate(RENORM_STEPS):
                    lf[m + 2:] += -np.log(BCS * np.float64(fd[mi, s]))
                logf.append(lf)
        out.append((Vg, logf))
    return out


def _cf(r, lf):
    # additive constant of alpha rows: alpha_r = log V_r + S_r*[r>0] + cf
    if r == 0:
        return B0
    return B0 + (r - 1) * MU + lf[r]


def stitch_direction(Vg, logf, sdir64):
    """-> delta [NCH] fp64, max stitch residual std (diagnostic)."""
    deltas = np.zeros(NCH)
    resid = 0.0
    prev_ovl = None
    delta = 0.0
    for g in range(NCH):
        warm = 0 if g == 0 else W
        lv = np.log(Vg[g].astype(np.float64) + 0.0)
        if g > 0:
            first = lv[warm] + sdir64[g * L] + _cf(warm, logf[g])
            dvec = prev_ovl - first
            delta = float(dvec.mean())
            resid = max(resid, float(dvec.std()))
        deltas[g] = delta
        if g + 1 < NCH:
            prev_ovl = (lv[warm + L] + sdir64[(g + 1) * L]
                        + _cf(warm + L, logf[g]) + delta)
    return deltas, resid


def host_stitch(res1, scores):
    s64 = scores.astype(np.float64)
    (Vf, lff), (Vb, lfb) = parse_scan_results(res1)
    df, rf = stitch_direction(Vf, lff, s64)
    db, rb = stitch_direction(Vb, lfb, s64[::-1])
    TIMINGS["stitch_resid"] = max(rf, rb)

    # Z from alpha row at position N-1 (last fwd chunk, r = W+L-1)
    g = NCH - 1
    aN = (np.log(Vf[g][W + L - 1].astype(np.float64)) + s64[N - 1]
          + _cf(W + L - 1, lff[g]) + df[g])
    m = aN.max()
    Z = m + np.log(np.exp(aN - m).sum())

    # per-position row constants
    i = np.arange(N)
    gf = i // L
    rfr = i - gf * L + np.where(gf > 0, W, 0)
    rev = N - 1 - i
    gb = rev // L
    rbr = rev - gb * L + np.where(gb > 0, W, 0)
    cf = np.empty(N)
    cb = np.empty(N)
    for k in range(N):
        cf[k] = _cf(rfr[k], lff[gf[k]]) + df[gf[k]]
        cb[k] = _cf(rbr[k], lfb[gb[k]]) + db[gb[k]]
    coef = ((rfr > 0).astype(np.float64) + (rbr > 0).astype(np.float64) - 1.0)
    rho = cf + cb - Z
    sp = (s64 * coef[:, None] + rho[:, None]).astype(np.float32)

    # gather V rows per position
    VF = np.empty((N, C), np.float32)
    VBr = np.empty((N, C), np.float32)
    for g in range(NCH):
        warm = 0 if g == 0 else W
        VF[g * L:(g + 1) * L] = Vf[g][warm:warm + L]
        VBr[g * L:(g + 1) * L] = Vb[g][warm:warm + L]
    VB = VBr[::-1]
    return VF, VB, sp


def prep_epi_inputs(VF, VB, sp):
    maps = []
    rows = N // NCORE
    for k in range(NCORE):
        sl = slice(k * rows, (k + 1) * rows)
        def lay(x):
            return np.ascontiguousarray(
                x[sl].reshape(rows // P, P, C).transpose(1, 0, 2)
                .reshape(P, rows * C // P))
        maps.append({"vf": lay(VF), "vb": lay(VB), "sp": lay(sp)})
    return maps


def assemble_output(res2):
    rows = N // NCORE
    out = np.empty((N, C), np.float32)
    for k in range(NCORE):
        o = res2[k]["out"].reshape(P, rows // P, C).transpose(1, 0, 2)
        out[k * rows:(k + 1) * rows] = o.reshape(rows, C)
    return out


# ---------------------------------------------------------------- emulation
def emulate_scan_core(inmap):
    tmat = inmap["tmat"]
    M16 = np.exp(tmat.astype(np.float32)).astype(np.float16)
    es = np.exp(inmap["srows"].astype(np.float32) - np.float32(MU))
    vst = np.zeros((P, R * NSC), np.float32)
    vst[:, 0:NSC] = es[:, 0:NSC] * np.float32(np.exp(MU - B0))
    u = vst[:, 0:NSC].astype(np.float16)
    fdump = np.ones((1, NREN * NS), np.float16)
    Mr = M16.astype(np.float32).reshape(P, CT, CT, P)   # [p, kt, jt, q]
    fbc = None
    for r in range(1, R):
        U = u.astype(np.float32).reshape(P, CT, NS)
        ps = np.einsum('pkjq,pks->qjs', Mr, U, optimize=True)
        ps = ps.reshape(P, NSC)
        vst[:, r * NSC:(r + 1) * NSC] = ps
        un = (ps * es[:, r * NSC:(r + 1) * NSC]).astype(np.float16)
        if r - 1 in RENORM_STEPS:
            un = (un.astype(np.float32) * fbc.astype(np.float32)).astype(np.float16)
        if r in RENORM_STEPS:
            ridx = RENORM_STEPS.index(r)
            sig = un.astype(np.float32).reshape(P, CT, NS).sum(axis=(0, 1))
            f16 = (np.float32(1.0) / sig).astype(np.float16)
            fdump[0, ridx * NS:(ridx + 1) * NS] = f16
            fb_row = (np.float32(BCS) * f16.astype(np.float32)).astype(np.float16)
            fbc = np.broadcast_to(np.tile(fb_row, CT)[None, :], (P, NSC))
        u = un
    return {"vdump": vst, "fdump": fdump}


def emulate_epi_core(inmap):
    m = inmap["vf"].astype(np.float32) * inmap["vb"].astype(np.float32)
    return {"out": np.log(m) + inmap["sp"]}


# ---------------------------------------------------------------- main entry
def kernel(scores, T, simulate=False):
    import time
    global _scan_nc, _epi_nc
    scores = np.ascontiguousarray(np.asarray(scores), dtype=np.float32)
    T = np.ascontiguousarray(np.asarray(T), dtype=np.float32)

    t0 = time.time()
    in1 = prep_scan_inputs(scores, T)
    TIMINGS["prep1"] = time.time() - t0

    t0 = time.time()
    if simulate:
        res1 = [emulate_scan_core(m) for m in in1]
    else:
        if _scan_nc is None:
            tb = time.time()
            _scan_nc = build_scan_nc()
            TIMINGS["build1"] = time.time() - tb
        res1 = run_bass_kernel_spmd(_scan_nc, in1, list(range(NCORE))).results
    TIMINGS["pass1"] = time.time() - t0

    t0 = time.time()
    VF, VB, sp = host_stitch(res1, scores)
    in2 = prep_epi_inputs(VF, VB, sp)
    TIMINGS["host"] = time.time() - t0

    t0 = time.time()
    if simulate:
        res2 = [emulate_epi_core(m) for m in in2]
    else:
        if _epi_nc is None:
            tb = time.time()
            _epi_nc = build_epi_nc()
            TIMINGS["build2"] = time.time() - tb
        res2 = run_bass_kernel_spmd(_epi_nc, in2, list(range(NCORE))).results
    TIMINGS["pass2"] = time.time() - t0

    t0 = time.time()
    out = assemble_output(res2)
    TIMINGS["asm"] = time.time() - t0
    return out



# revision 3
# speedup vs baseline: 2.4239x; 2.4239x over previous
# Linear-chain CRF log-marginals on 8 Trainium2 NeuronCores.
#
# alpha/beta recurrences run in the exp domain: the per-step
# LSE_k(alpha[k] + T[k,j]) becomes a matvec u @ exp(T) on the PE array
# (fp16 operands, fp32 PSUM accumulate), with a constant per-step prescale
# exp(-MU) folded into exp(scores).  The sequence is split into many short
# chunks run speculatively in lockstep (the chain mixes ~15x per step, so a
# W=2 warmup makes each chunk's carry exact up to a common constant);
# 128 chunk-scans per core give the matmuls a 128-wide free dim so the
# per-(kt,jt) weight reload is hidden under the streaming.  ln(V) is
# computed on ScalarE straight from PSUM and dumped in fp16; scores ship
# as bf16.  Chunk constants are resolved on the host in fp64 by matching
# one overlap row per boundary.  A second tiny device pass adds
# lnVf + lnVb + coef*scores + rho for the final marginals.
import numpy as np
from contextlib import ExitStack

import ml_dtypes

import concourse.bass as bass
import concourse.tile as tile
from concourse import bacc, mybir
from concourse.bass_utils import run_bass_kernel_spmd

F16 = mybir.dt.float16
F32 = mybir.dt.float32
BF16 = mybir.dt.bfloat16
AFT = mybir.ActivationFunctionType
BFNP = ml_dtypes.bfloat16

# ---- problem constants ----
N, C = 8192, 1024
P = 128                  # partitions
CT = C // P              # 8 contraction/output tiles of 128 classes
NCORE = 8

# ---- algorithm parameters ----
NS = 128                 # lockstep scans per core
NCH = 4 * NS             # 512 chunks per direction (4 cores each direction)
L = N // NCH             # 16 positions per chunk
W = 2                    # warmup steps per speculative chunk
R = W + L + 1            # rows per scan: init row + W+L steps
MU = 7.927               # constant per-step log-prescale
B0 = 4.0                 # init offset: u_0 = exp(s_0 - B0)
G = 7                    # rows per DMA/exp group
NSC = CT * NS            # 1024 carry columns per core
HNS = 4 * NS             # half of the carry (one PSUM bank)

_scan_nc = None
TIMINGS = {}


# ---------------------------------------------------------------- builders
def build_scan_nc(steps=None, repeats=1, timing_loop=None):
    nsteps = R if steps is None else steps
    nc = bacc.Bacc(None, target_bir_lowering=False)
    m16d = nc.declare_dram_parameter("m16", [P, C * CT], F16, isOutput=False)
    srows = nc.declare_dram_parameter("srows", [P, R * NSC], BF16, isOutput=False)
    vdump = nc.declare_dram_parameter("vdump", [P, R * NSC], F16, isOutput=True)

    ngroups = (R + G - 1) // G

    with tile.TileContext(nc) as tc, ExitStack() as ctx:
        mpool = ctx.enter_context(tc.tile_pool(name="m16", bufs=1))
        espool = ctx.enter_context(tc.tile_pool(name="es", bufs=2))
        vpool = ctx.enter_context(tc.tile_pool(name="vst", bufs=2))
        upool = ctx.enter_context(tc.tile_pool(name="u", bufs=4))
        psA = ctx.enter_context(tc.tile_pool(name="psA", bufs=2, space="PSUM"))
        psB = ctx.enter_context(tc.tile_pool(name="psB", bufs=2, space="PSUM"))

        m16 = mpool.tile([P, C * CT], F16)
        nc.sync.dma_start(m16[:], m16d[:])

        es_tiles = [None] * ngroups

        def emit_group(g):
            lo = g * G * NSC
            hi = min(R, (g + 1) * G) * NSC
            et = espool.tile([P, G * NSC], BF16)
            nc.sync.dma_start(et[:, 0:hi - lo], srows[:, lo:hi])
            es_tiles[g] = et

        loop_cm = tc.For_i(0, timing_loop, 1) if timing_loop else ExitStack()
        with loop_cm:
            for g in range(ngroups):
                es_tiles[g] = None
            emit_group(0)

            vst = vpool.tile([P, G * NSC], F16)
            # r = 0: lnV0 = ln(es0 * e^(MU-B0)) = s0 - B0
            nc.scalar.activation(vst[:, 0:NSC], es_tiles[0][:, 0:NSC],
                                 AFT.Ln, scale=float(np.exp(MU - B0)))
            uA = upool.tile([P, HNS], F16)
            uB = upool.tile([P, HNS], F16)
            nc.scalar.mul(uA[:], es_tiles[0][:, 0:HNS], float(np.exp(MU - B0)))
            nc.scalar.mul(uB[:], es_tiles[0][:, HNS:NSC], float(np.exp(MU - B0)))

            for r in range(1, nsteps):
                g, slot = divmod(r, G)
                if slot == 0:
                    if es_tiles[g] is None:
                        emit_group(g)
                    vst = vpool.tile([P, G * NSC], F16)
                if slot == 0 and g + 1 < ngroups and es_tiles[g + 1] is None:
                    emit_group(g + 1)
                es = es_tiles[g]
                off = slot * NSC

                psa = psA.tile([P, HNS], F32)
                psb = psB.tile([P, HNS], F32)
                # MM order: both uA-halves of psa, psb... grouped so psa
                # finishes at MM32 and the first 32 MMs only need uA.
                for half in range(2):            # 0: kt 0-3 (uA), 1: kt 4-7 (uB)
                    u_src = uA if half == 0 else uB
                    for jh in range(2):          # 0: jt 0-3 (psa), 1: jt 4-7 (psb)
                        tgt = psa if jh == 0 else psb
                        for kq in range(4):
                            kt = half * 4 + kq
                            for jq in range(4):
                                jt = jh * 4 + jq
                                nc.tensor.matmul(
                                    tgt[:, jq * NS:(jq + 1) * NS],
                                    m16[:, (kt * CT + jt) * P:(kt * CT + jt + 1) * P],
                                    u_src[:, kq * NS:(kq + 1) * NS],
                                    start=(half == 0 and kq == 0 and jq == 0),
                                    stop=(half == 1 and kq == 3 and jq == 3),
                                )
                # u-carry first (critical path: Tile serializes same-PSUM
                # readers in program order), ln(V) dump second
                if r < nsteps - 1:
                    uA = upool.tile([P, HNS], F16)
                    uB = upool.tile([P, HNS], F16)
                    nc.vector.tensor_mul(uA[:], psa[:], es[:, off:off + HNS])
                    nc.vector.tensor_mul(uB[:], psb[:], es[:, off + HNS:off + NSC])
                nc.scalar.activation(vst[:, off:off + HNS], psa[:], AFT.Ln)
                nc.scalar.activation(vst[:, off + HNS:off + NSC], psb[:], AFT.Ln)
                if slot == G - 1 or r == nsteps - 1:
                    lo = g * G * NSC
                    hi = min(R, (g + 1) * G) * NSC
                    nc.sync.dma_start(vdump[:, lo:hi], vst[:, 0:hi - lo])
    nc.finalize()
    return nc


# ---------------------------------------------------------------- host prep
def build_chunk_scores(sdir):
    """Per-direction chunk score rows [NCH, R, C] (fp32, zero-padded)."""
    SS = np.zeros((NCH, R, C), np.float32)
    for g in range(NCH):
        lo = 0 if g == 0 else g * L - W
        seg = sdir[lo:min(lo + R, N)]
        SS[g, :seg.shape[0]] = seg
    return SS


def prep_scan_inputs(scores, T):
    maps = []
    for d, (Tp, sdir) in enumerate([(T, scores), (T.T, scores[::-1])]):
        tmat = np.ascontiguousarray(
            Tp.reshape(P, CT, P, CT).transpose(0, 1, 3, 2).reshape(P, CT * CT * P),
            dtype=np.float32)
        m16 = np.exp(tmat).astype(np.float16)
        SS = build_chunk_scores(sdir)
        np.exp(SS - np.float32(MU), out=SS)              # es rows, in place
        for cidx in range(4):
            SSc = SS[cidx * NS:(cidx + 1) * NS]          # [NS, R, C]
            srows = np.ascontiguousarray(
                SSc.reshape(NS, R, P, CT).transpose(2, 1, 3, 0)
                .reshape(P, R * NSC)).astype(BFNP)
            maps.append({"m16": m16, "srows": srows})
    return maps


def parse_scan_results(res):
    """-> per direction: list of lnV [NCH][R, C] fp16."""
    out = []
    for d in range(2):
        Vg = []
        for cidx in range(4):
            vd = res[d * 4 + cidx]["vdump"].reshape(P, R, CT, NS)
            arr = np.ascontiguousarray(
                vd.transpose(3, 1, 0, 2).reshape(NS, R, C))
            for s in range(NS):
                Vg.append(arr[s])
        out.append(Vg)
    return out


def _cf(r):
    # additive constant of alpha rows: alpha_r = lnV_r + S_r*[r>0] + cf
    return B0 if r == 0 else B0 + (r - 1) * MU


def _cf_vec(r):
    return np.where(r == 0, B0, B0 + (r - 1.0) * MU)


def stitch_direction(Vg, sdir64):
    """-> delta [NCH] fp64, max stitch residual std (diagnostic)."""
    deltas = np.zeros(NCH)
    resid = 0.0
    prev_ovl = None
    delta = 0.0
    for g in range(NCH):
        warm = 0 if g == 0 else W
        lv = Vg[g]
        if g > 0:
            first = lv[warm].astype(np.float64) + sdir64[g * L] + _cf(warm)
            dvec = prev_ovl - first
            delta = float(dvec.mean())
            resid = max(resid, float(dvec.std()))
        deltas[g] = delta
        if g + 1 < NCH:
            prev_ovl = (lv[warm + L].astype(np.float64) + sdir64[(g + 1) * L]
                        + _cf(warm + L) + delta)
    return deltas, resid


def host_stitch(res1, scores):
    s64 = scores.astype(np.float64)
    Vf, Vb = parse_scan_results(res1)
    df, rf = stitch_direction(Vf, s64)
    db, rb = stitch_direction(Vb, s64[::-1])
    TIMINGS["stitch_resid"] = max(rf, rb)

    # Z from alpha row at position N-1 (last fwd chunk, r = W+L-1)
    g = NCH - 1
    aN = (Vf[g][W + L - 1].astype(np.float64) + s64[N - 1]
          + _cf(W + L - 1) + df[g])
    m = aN.max()
    Z = m + np.log(np.exp(aN - m).sum())

    # per-position row constants
    i = np.arange(N)
    gf = i // L
    rfr = i - gf * L + np.where(gf > 0, W, 0)
    rev = N - 1 - i
    gb = rev // L
    rbr = rev - gb * L + np.where(gb > 0, W, 0)
    cf = _cf_vec(rfr) + df[gf]
    cb = _cf_vec(rbr) + db[gb]
    coef = ((rfr > 0).astype(np.float64) + (rbr > 0).astype(np.float64) - 1.0)
    rho = (cf + cb - Z).astype(np.float32)
    sp2 = (s64 * coef[:, None]).astype(np.float32)

    # gather lnV rows per position; fold the scores term into the fwd half
    LF = np.empty((N, C), np.float16)
    LBr = np.empty((N, C), np.float16)
    for g in range(NCH):
        warm = 0 if g == 0 else W
        LF[g * L:(g + 1) * L] = (
            Vf[g][warm:warm + L].astype(np.float32) + sp2[g * L:(g + 1) * L])
        LBr[g * L:(g + 1) * L] = Vb[g][warm:warm + L]
    LB = LBr[::-1]
    return LF, LB, rho


# ---------------------------------------------------------------- emulation
def emulate_scan_core(inmap):
    M16 = inmap["m16"]
    es = inmap["srows"].astype(np.float32)
    vst = np.zeros((P, R * NSC), np.float16)
    vst[:, 0:NSC] = np.log(
        es[:, 0:NSC] * np.float32(np.exp(MU - B0))).astype(np.float16)
    u = (es[:, 0:NSC] * np.float32(np.exp(MU - B0))).astype(np.float16)
    Mr = M16.astype(np.float32).reshape(P, CT, CT, P)   # [p, kt, jt, q]
    for r in range(1, R):
        U = u.astype(np.float32).reshape(P, CT, NS)
        ps = np.einsum('pkjq,pks->qjs', Mr, U, optimize=True)
        ps = ps.reshape(P, NSC)
        vst[:, r * NSC:(r + 1) * NSC] = np.log(ps).astype(np.float16)
        if r < R - 1:
            u = (ps * es[:, r * NSC:(r + 1) * NSC]).astype(np.float16)
    return {"vdump": vst}


# ---------------------------------------------------------------- main entry
def kernel(scores, T, simulate=False):
    import time
    global _scan_nc
    scores = np.ascontiguousarray(np.asarray(scores), dtype=np.float32)
    T = np.ascontiguousarray(np.asarray(T), dtype=np.float32)

    t0 = time.time()
    in1 = prep_scan_inputs(scores, T)
    TIMINGS["prep1"] = time.time() - t0

    t0 = time.time()
    if simulate:
        res1 = [emulate_scan_core(m) for m in in1]
    else:
        if _scan_nc is None:
            tb = time.time()
            _scan_nc = build_scan_nc()
            TIMINGS["build1"] = time.time() - tb
        res1 = run_bass_kernel_spmd(_scan_nc, in1, list(range(NCORE))).results
    TIMINGS["pass1"] = time.time() - t0

    t0 = time.time()
    LF, LB, rho = host_stitch(res1, scores)
    # final combine: alpha + beta - scores - Z, all constants folded in
    out = LF.astype(np.float32)
    out += LB.astype(np.float32)
    out += rho[:, None]
    TIMINGS["host"] = time.time() - t0
    return out


# revision 4
# speedup vs baseline: 3.9136x; 1.6146x over previous
# Linear-chain CRF log-marginals on 8 Trainium2 NeuronCores.
#
# alpha/beta recurrences run in the exp domain: the per-step
# LSE_k(alpha[k] + T[k,j]) becomes a matvec u @ exp(T) on the PE array
# (fp16 operands, fp32 PSUM accumulate), with a constant per-step prescale
# exp(-MU) folded into exp(scores).  The sequence is split into many short
# chunks run speculatively in lockstep (the chain mixes ~15x per step, so a
# W=2 warmup makes each chunk's carry exact up to a common constant);
# 128 chunk-scans per core give the matmuls a 128-wide free dim so the
# per-(kt,jt) weight reload is hidden under the streaming.  ln(V) is
# computed on ScalarE straight from PSUM and dumped in fp16; scores ship
# as bf16.  Chunk constants are resolved on the host in fp64 by matching
# one overlap row per boundary.  A second tiny device pass adds
# lnVf + lnVb + coef*scores + rho for the final marginals.
import numpy as np
from contextlib import ExitStack

import ml_dtypes

import concourse.bass as bass
import concourse.tile as tile
from concourse import bacc, mybir
from concourse.bass_utils import run_bass_kernel_spmd

F16 = mybir.dt.float16
F32 = mybir.dt.float32
BF16 = mybir.dt.bfloat16
F8 = mybir.dt.float8e4
AFT = mybir.ActivationFunctionType
BFNP = ml_dtypes.bfloat16
F8NP = ml_dtypes.float8_e4m3

# ---- problem constants ----
N, C = 8192, 1024
P = 128                  # partitions
CT = C // P              # 8 contraction/output tiles of 128 classes
NCORE = 8

# ---- algorithm parameters ----
NS = 128                 # lockstep scans per core
NCH = 4 * NS             # 512 chunks per direction (4 cores each direction)
L = N // NCH             # 16 positions per chunk
W = 1                    # warmup steps per speculative chunk
R = W + L + 1            # rows per scan: init row + W+L steps
MU = 7.927               # constant per-step log-prescale
B0 = 4.0                 # init offset: u_0 = exp(s_0 - B0)
G = 7                    # rows per DMA/exp group
NSC = CT * NS            # 1024 carry columns per core
HNS = 4 * NS             # half of the carry (one PSUM bank)

_scan_nc = None
TIMINGS = {}


# ---------------------------------------------------------------- builders
def build_scan_nc(steps=None, repeats=1, timing_loop=None):
    nsteps = R if steps is None else steps
    nc = bacc.Bacc(None, target_bir_lowering=False)
    m16d = nc.declare_dram_parameter("m16", [P, C * CT], F8, isOutput=False)
    srows = nc.declare_dram_parameter("srows", [P, R * NSC], BF16, isOutput=False)
    vdump = nc.declare_dram_parameter("vdump", [P, R * NSC], F16, isOutput=True)

    ngroups = (R + G - 1) // G

    with tile.TileContext(nc) as tc, ExitStack() as ctx:
        mpool = ctx.enter_context(tc.tile_pool(name="m16", bufs=1))
        espool = ctx.enter_context(tc.tile_pool(name="es", bufs=2))
        vpool = ctx.enter_context(tc.tile_pool(name="vst", bufs=2))
        upool = ctx.enter_context(tc.tile_pool(name="u", bufs=4))
        psA = ctx.enter_context(tc.tile_pool(name="psA", bufs=2, space="PSUM"))
        psB = ctx.enter_context(tc.tile_pool(name="psB", bufs=2, space="PSUM"))

        m16 = mpool.tile([P, C * CT], F8)
        nc.sync.dma_start(m16[:], m16d[:])

        es_tiles = [None] * ngroups

        def emit_group(g):
            lo = g * G * NSC
            hi = min(R, (g + 1) * G) * NSC
            et = espool.tile([P, G * NSC], BF16)
            nc.sync.dma_start(et[:, 0:hi - lo], srows[:, lo:hi])
            es_tiles[g] = et

        loop_cm = tc.For_i(0, timing_loop, 1) if timing_loop else ExitStack()
        with loop_cm:
            for g in range(ngroups):
                es_tiles[g] = None
            emit_group(0)
            if ngroups > 1:
                emit_group(1)

            vst = vpool.tile([P, G * NSC], F16)
            # r = 0: lnV0 = ln(es0 * e^(MU-B0)) = s0 - B0
            nc.scalar.activation(vst[:, 0:NSC], es_tiles[0][:, 0:NSC],
                                 AFT.Ln, scale=float(np.exp(MU - B0)))
            uA = upool.tile([P, HNS], F16)
            uB = upool.tile([P, HNS], F16)
            nc.scalar.mul(uA[:], es_tiles[0][:, 0:HNS], float(np.exp(MU - B0)))
            nc.scalar.mul(uB[:], es_tiles[0][:, HNS:NSC], float(np.exp(MU - B0)))

            for r in range(1, nsteps):
                g, slot = divmod(r, G)
                if slot == 0:
                    if es_tiles[g] is None:
                        emit_group(g)
                    vst = vpool.tile([P, G * NSC], F16)
                if slot == 0 and g + 1 < ngroups and es_tiles[g + 1] is None:
                    emit_group(g + 1)
                es = es_tiles[g]
                off = slot * NSC

                psa = psA.tile([P, HNS], F32)
                psb = psB.tile([P, HNS], F32)
                # MM order: both uA-halves of psa, psb... grouped so psa
                # finishes at MM32 and the first 32 MMs only need uA.
                for half in range(2):            # 0: kt 0-3 (uA), 1: kt 4-7 (uB)
                    u_src = uA if half == 0 else uB
                    for jh in range(2):          # 0: jt 0-3 (psa), 1: jt 4-7 (psb)
                        tgt = psa if jh == 0 else psb
                        for kq in range(4):
                            kt = half * 4 + kq
                            for jq in range(4):
                                jt = jh * 4 + jq
                                nc.tensor.matmul(
                                    tgt[:, jq * NS:(jq + 1) * NS],
                                    m16[:, (kt * CT + jt) * P:(kt * CT + jt + 1) * P],
                                    u_src[:, kq * NS:(kq + 1) * NS],
                                    start=(half == 0 and kq == 0 and jq == 0),
                                    stop=(half == 1 and kq == 3 and jq == 3),
                                )
                # u-carry first (critical path: Tile serializes same-PSUM
                # readers in program order), ln(V) dump second
                if r < nsteps - 1:
                    uA = upool.tile([P, HNS], F16)
                    uB = upool.tile([P, HNS], F16)
                    nc.vector.tensor_mul(uA[:], psa[:], es[:, off:off + HNS])
                    nc.vector.tensor_mul(uB[:], psb[:], es[:, off + HNS:off + NSC])
                nc.scalar.activation(vst[:, off:off + HNS], psa[:], AFT.Ln)
                nc.scalar.activation(vst[:, off + HNS:off + NSC], psb[:], AFT.Ln)
                if slot == G - 1 or r == nsteps - 1:
                    lo = g * G * NSC
                    hi = min(R, (g + 1) * G) * NSC
                    nc.sync.dma_start(vdump[:, lo:hi], vst[:, 0:hi - lo])
    nc.finalize()
    return nc


# ---------------------------------------------------------------- host prep
def build_chunk_scores(sdir):
    """Per-direction chunk score rows [NCH, R, C] (fp32, zero-padded)."""
    SS = np.zeros((NCH, R, C), np.float32)
    for g in range(NCH):
        lo = 0 if g == 0 else g * L - W
        seg = sdir[lo:min(lo + R, N)]
        SS[g, :seg.shape[0]] = seg
    return SS


def prep_scan_inputs(scores, T):
    maps = []
    for d, (Tp, sdir) in enumerate([(T, scores), (T.T, scores[::-1])]):
        tmat = np.ascontiguousarray(
            Tp.reshape(P, CT, P, CT).transpose(0, 1, 3, 2).reshape(P, CT * CT * P),
            dtype=np.float32)
        m16 = np.exp(tmat).astype(F8NP)
        SS = build_chunk_scores(sdir)
        np.exp(SS - np.float32(MU), out=SS)              # es rows, in place
        for cidx in range(4):
            SSc = SS[cidx * NS:(cidx + 1) * NS]          # [NS, R, C]
            srows = np.ascontiguousarray(
                SSc.reshape(NS, R, P, CT).transpose(2, 1, 3, 0)
                .reshape(P, R * NSC)).astype(BFNP)
            maps.append({"m16": m16, "srows": srows})
    return maps


def parse_scan_results(res):
    """-> per direction: list of lnV [NCH][R, C] fp16."""
    out = []
    for d in range(2):
        Vg = []
        for cidx in range(4):
            vd = res[d * 4 + cidx]["vdump"].reshape(P, R, CT, NS)
            arr = np.ascontiguousarray(
                vd.transpose(3, 1, 0, 2).reshape(NS, R, C))
            for s in range(NS):
                Vg.append(arr[s])
        out.append(Vg)
    return out


def _cf(r):
    # additive constant of alpha rows: alpha_r = lnV_r + S_r*[r>0] + cf
    return B0 if r == 0 else B0 + (r - 1) * MU


def _cf_vec(r):
    return np.where(r == 0, B0, B0 + (r - 1.0) * MU)


def stitch_direction(Vg, sdir64):
    """-> delta [NCH] fp64, max stitch residual std (diagnostic)."""
    deltas = np.zeros(NCH)
    resid = 0.0
    prev_ovl = None
    delta = 0.0
    for g in range(NCH):
        warm = 0 if g == 0 else W
        lv = Vg[g]
        if g > 0:
            first = lv[warm].astype(np.float64) + sdir64[g * L] + _cf(warm)
            dvec = prev_ovl - first
            delta = float(dvec.mean())
            resid = max(resid, float(dvec.std()))
        deltas[g] = delta
        if g + 1 < NCH:
            prev_ovl = (lv[warm + L].astype(np.float64) + sdir64[(g + 1) * L]
                        + _cf(warm + L) + delta)
    return deltas, resid


def host_stitch(res1, scores):
    s64 = scores.astype(np.float64)
    Vf, Vb = parse_scan_results(res1)
    df, rf = stitch_direction(Vf, s64)
    db, rb = stitch_direction(Vb, s64[::-1])
    TIMINGS["stitch_resid"] = max(rf, rb)

    # Z from alpha row at position N-1 (last fwd chunk, r = W+L-1)
    g = NCH - 1
    aN = (Vf[g][W + L - 1].astype(np.float64) + s64[N - 1]
          + _cf(W + L - 1) + df[g])
    m = aN.max()
    Z = m + np.log(np.exp(aN - m).sum())

    # per-position row constants
    i = np.arange(N)
    gf = i // L
    rfr = i - gf * L + np.where(gf > 0, W, 0)
    rev = N - 1 - i
    gb = rev // L
    rbr = rev - gb * L + np.where(gb > 0, W, 0)
    cf = _cf_vec(rfr) + df[gf]
    cb = _cf_vec(rbr) + db[gb]
    coef = ((rfr > 0).astype(np.float64) + (rbr > 0).astype(np.float64) - 1.0)
    rho = (cf + cb - Z).astype(np.float32)
    sp2 = (s64 * coef[:, None]).astype(np.float32)

    # gather lnV rows per position; fold the scores term into the fwd half
    LF = np.empty((N, C), np.float16)
    LBr = np.empty((N, C), np.float16)
    for g in range(NCH):
        warm = 0 if g == 0 else W
        LF[g * L:(g + 1) * L] = (
            Vf[g][warm:warm + L].astype(np.float32) + sp2[g * L:(g + 1) * L])
        LBr[g * L:(g + 1) * L] = Vb[g][warm:warm + L]
    LB = LBr[::-1]
    return LF, LB, rho


# ---------------------------------------------------------------- emulation
def emulate_scan_core(inmap):
    M16 = inmap["m16"]
    es = inmap["srows"].astype(np.float32)
    vst = np.zeros((P, R * NSC), np.float16)
    vst[:, 0:NSC] = np.log(
        es[:, 0:NSC] * np.float32(np.exp(MU - B0))).astype(np.float16)
    u = (es[:, 0:NSC] * np.float32(np.exp(MU - B0))).astype(np.float16)
    Mr = M16.astype(np.float32).reshape(P, CT, CT, P)   # [p, kt, jt, q]
    for r in range(1, R):
        U = u.astype(np.float32).reshape(P, CT, NS)
        ps = np.einsum('pkjq,pks->qjs', Mr, U, optimize=True)
        ps = ps.reshape(P, NSC)
        vst[:, r * NSC:(r + 1) * NSC] = np.log(ps).astype(np.float16)
        if r < R - 1:
            u = (ps * es[:, r * NSC:(r + 1) * NSC]).astype(np.float16)
    return {"vdump": vst}


# ---------------------------------------------------------------- main entry
def kernel(scores, T, simulate=False):
    import time
    global _scan_nc
    scores = np.ascontiguousarray(np.asarray(scores), dtype=np.float32)
    T = np.ascontiguousarray(np.asarray(T), dtype=np.float32)

    t0 = time.time()
    in1 = prep_scan_inputs(scores, T)
    TIMINGS["prep1"] = time.time() - t0

    t0 = time.time()
    if simulate:
        res1 = [emulate_scan_core(m) for m in in1]
    else:
        if _scan_nc is None:
            tb = time.time()
            _scan_nc = build_scan_nc()
            TIMINGS["build1"] = time.time() - tb
        res1 = run_bass_kernel_spmd(_scan_nc, in1, list(range(NCORE))).results
    TIMINGS["pass1"] = time.time() - t0

    t0 = time.time()
    LF, LB, rho = host_stitch(res1, scores)
    # final combine: alpha + beta - scores - Z, all constants folded in
    out = LF.astype(np.float32)
    out += LB.astype(np.float32)
    out += rho[:, None]
    TIMINGS["host"] = time.time() - t0
    return out


# revision 5
# speedup vs baseline: 4.0931x; 1.0458x over previous
# Linear-chain CRF log-marginals on 8 Trainium2 NeuronCores.
#
# alpha/beta recurrences run in the exp domain: the per-step
# LSE_k(alpha[k] + T[k,j]) becomes a matvec u @ exp(T) on the PE array
# (fp16 operands, fp32 PSUM accumulate), with a constant per-step prescale
# exp(-MU) folded into exp(scores).  The sequence is split into many short
# chunks run speculatively in lockstep (the chain mixes ~15x per step, so a
# W=2 warmup makes each chunk's carry exact up to a common constant);
# 128 chunk-scans per core give the matmuls a 128-wide free dim so the
# per-(kt,jt) weight reload is hidden under the streaming.  ln(V) is
# computed on ScalarE straight from PSUM and dumped in fp16; scores ship
# as bf16.  Chunk constants are resolved on the host in fp64 by matching
# one overlap row per boundary.  A second tiny device pass adds
# lnVf + lnVb + coef*scores + rho for the final marginals.
import numpy as np
from contextlib import ExitStack

import ml_dtypes

import concourse.bass as bass
import concourse.tile as tile
from concourse import bacc, mybir
from concourse.bass_utils import run_bass_kernel_spmd

F16 = mybir.dt.float16
F32 = mybir.dt.float32
BF16 = mybir.dt.bfloat16
F8 = mybir.dt.float8e4
AFT = mybir.ActivationFunctionType
BFNP = ml_dtypes.bfloat16
F8NP = ml_dtypes.float8_e4m3

# ---- problem constants ----
N, C = 8192, 1024
P = 128                  # partitions
CT = C // P              # 8 contraction/output tiles of 128 classes
NCORE = 8

# ---- algorithm parameters ----
NS = 128                 # lockstep scans per core
NCH = 4 * NS             # 512 chunks per direction (4 cores each direction)
L = N // NCH             # 16 positions per chunk
W = 1                    # warmup steps per speculative chunk
R = W + L + 1            # rows per scan: init row + W+L steps
MU = 7.927               # constant per-step log-prescale
B0 = 4.0                 # init offset: u_0 = exp(s_0 - B0)
G = 7                    # rows per DMA/exp group
NSC = CT * NS            # 1024 carry columns per core
HNS = 4 * NS             # half of the carry (one PSUM bank)

_scan_nc = None
TIMINGS = {}


# ---------------------------------------------------------------- builders
def build_scan_nc(steps=None, repeats=1, timing_loop=None):
    nsteps = R if steps is None else steps
    nc = bacc.Bacc(None, target_bir_lowering=False)
    m16d = nc.declare_dram_parameter("m16", [P, C * CT], F8, isOutput=False)
    srows = nc.declare_dram_parameter("srows", [P, R * NSC], BF16, isOutput=False)
    vdump = nc.declare_dram_parameter("vdump", [P, R * NSC], F16, isOutput=True)

    ngroups = (R + G - 1) // G

    with tile.TileContext(nc) as tc, ExitStack() as ctx:
        mpool = ctx.enter_context(tc.tile_pool(name="m16", bufs=1))
        espool = ctx.enter_context(tc.tile_pool(name="es", bufs=3))
        vpool = ctx.enter_context(tc.tile_pool(name="vst", bufs=3))
        upool = ctx.enter_context(tc.tile_pool(name="u", bufs=3))
        psA = ctx.enter_context(tc.tile_pool(name="psA", bufs=2, space="PSUM"))
        psB = ctx.enter_context(tc.tile_pool(name="psB", bufs=2, space="PSUM"))

        m16 = mpool.tile([P, C * CT], F8)
        nc.sync.dma_start(m16[:], m16d[:])

        es_tiles = [None] * ngroups

        def emit_group(g):
            lo = g * G * NSC
            hi = min(R, (g + 1) * G) * NSC
            et = espool.tile([P, G * NSC], BF16)
            nc.sync.dma_start(et[:, 0:hi - lo], srows[:, lo:hi])
            es_tiles[g] = et

        loop_cm = tc.For_i(0, timing_loop, 1) if timing_loop else ExitStack()
        with loop_cm:
            for g in range(ngroups):
                es_tiles[g] = None
            emit_group(0)
            if ngroups > 1:
                emit_group(1)

            vst = vpool.tile([P, G * NSC], F16)
            # r = 0: lnV0 = ln(es0 * e^(MU-B0)) = s0 - B0
            nc.scalar.activation(vst[:, 0:NSC], es_tiles[0][:, 0:NSC],
                                 AFT.Ln, scale=float(np.exp(MU - B0)))
            u_prev = [upool.tile([P, NS], F16, name=f'u_{ct}') for ct in range(CT)]
            for ct in range(CT):
                nc.scalar.mul(u_prev[ct][:], es_tiles[0][:, ct * NS:(ct + 1) * NS],
                              float(np.exp(MU - B0)))

            for r in range(1, nsteps):
                g, slot = divmod(r, G)
                if slot == 0:
                    if es_tiles[g] is None:
                        emit_group(g)
                    vst = vpool.tile([P, G * NSC], F16)
                if slot == 0 and g + 1 < ngroups and es_tiles[g + 1] is None:
                    emit_group(g + 1)
                es = es_tiles[g]
                off = slot * NSC

                psa = psA.tile([P, HNS], F32)
                psb = psB.tile([P, HNS], F32)
                # MM order: first 32 MMs consume only u_prev[0:4], so the
                # carry update of the previous step has a full half-step of
                # cover.  Each psa/psb region's final (kt=7) write lands in
                # the last 8 MMs; the per-region muls below start as soon as
                # their region stops.
                for half in range(2):            # 0: kt 0-3, 1: kt 4-7
                    for jh in range(2):          # 0: jt 0-3 (psa), 1: jt 4-7 (psb)
                        tgt = psa if jh == 0 else psb
                        for kq in range(4):
                            kt = half * 4 + kq
                            for jq in range(4):
                                jt = jh * 4 + jq
                                nc.tensor.matmul(
                                    tgt[:, jq * NS:(jq + 1) * NS],
                                    m16[:, (kt * CT + jt) * P:(kt * CT + jt + 1) * P],
                                    u_prev[kt][:],
                                    start=(half == 0 and kq == 0 and jq == 0),
                                    stop=(half == 1 and kq == 3 and jq == 3),
                                )
                # per-region u-carry first (critical path: Tile serializes
                # same-PSUM readers in program order), ln(V) dump second
                if r < nsteps - 1:
                    u_nxt = [upool.tile([P, NS], F16, name=f'u_{ct}') for ct in range(CT)]
                    for ct in range(CT):
                        src = psa if ct < 4 else psb
                        q = ct % 4
                        nc.vector.tensor_mul(
                            u_nxt[ct][:], src[:, q * NS:(q + 1) * NS],
                            es[:, off + ct * NS:off + (ct + 1) * NS])
                    u_prev = u_nxt
                nc.scalar.activation(vst[:, off:off + HNS], psa[:], AFT.Ln)
                nc.scalar.activation(vst[:, off + HNS:off + NSC], psb[:], AFT.Ln)
                if slot == G - 1 or r == nsteps - 1:
                    lo = g * G * NSC
                    hi = min(R, (g + 1) * G) * NSC
                    # store on the Activation DGE ring so the next es load
                    # (SP ring) never queues behind it
                    nc.scalar.dma_start(vdump[:, lo:hi], vst[:, 0:hi - lo])
    nc.finalize()
    return nc


# ---------------------------------------------------------------- host prep
def build_chunk_scores(sdir):
    """Per-direction chunk score rows [NCH, R, C] (fp32, zero-padded)."""
    SS = np.zeros((NCH, R, C), np.float32)
    for g in range(NCH):
        lo = 0 if g == 0 else g * L - W
        seg = sdir[lo:min(lo + R, N)]
        SS[g, :seg.shape[0]] = seg
    return SS


def prep_scan_inputs(scores, T):
    maps = []
    for d, (Tp, sdir) in enumerate([(T, scores), (T.T, scores[::-1])]):
        tmat = np.ascontiguousarray(
            Tp.reshape(P, CT, P, CT).transpose(0, 1, 3, 2).reshape(P, CT * CT * P),
            dtype=np.float32)
        m16 = np.exp(tmat).astype(F8NP)
        SS = build_chunk_scores(sdir)
        np.exp(SS - np.float32(MU), out=SS)              # es rows, in place
        for cidx in range(4):
            SSc = SS[cidx * NS:(cidx + 1) * NS]          # [NS, R, C]
            srows = np.ascontiguousarray(
                SSc.reshape(NS, R, P, CT).transpose(2, 1, 3, 0)
                .reshape(P, R * NSC)).astype(BFNP)
            maps.append({"m16": m16, "srows": srows})
    return maps


def parse_scan_results(res):
    """-> per direction: list of lnV [NCH][R, C] fp16."""
    out = []
    for d in range(2):
        Vg = []
        for cidx in range(4):
            vd = res[d * 4 + cidx]["vdump"].reshape(P, R, CT, NS)
            arr = np.ascontiguousarray(
                vd.transpose(3, 1, 0, 2).reshape(NS, R, C))
            for s in range(NS):
                Vg.append(arr[s])
        out.append(Vg)
    return out


def _cf(r):
    # additive constant of alpha rows: alpha_r = lnV_r + S_r*[r>0] + cf
    return B0 if r == 0 else B0 + (r - 1) * MU


def _cf_vec(r):
    return np.where(r == 0, B0, B0 + (r - 1.0) * MU)


def stitch_direction(Vg, sdir64):
    """-> delta [NCH] fp64, max stitch residual std (diagnostic)."""
    deltas = np.zeros(NCH)
    resid = 0.0
    prev_ovl = None
    delta = 0.0
    for g in range(NCH):
        warm = 0 if g == 0 else W
        lv = Vg[g]
        if g > 0:
            first = lv[warm].astype(np.float64) + sdir64[g * L] + _cf(warm)
            dvec = prev_ovl - first
            delta = float(dvec.mean())
            resid = max(resid, float(dvec.std()))
        deltas[g] = delta
        if g + 1 < NCH:
            prev_ovl = (lv[warm + L].astype(np.float64) + sdir64[(g + 1) * L]
                        + _cf(warm + L) + delta)
    return deltas, resid


def host_stitch(res1, scores):
    s64 = scores.astype(np.float64)
    Vf, Vb = parse_scan_results(res1)
    df, rf = stitch_direction(Vf, s64)
    db, rb = stitch_direction(Vb, s64[::-1])
    TIMINGS["stitch_resid"] = max(rf, rb)

    # Z from alpha row at position N-1 (last fwd chunk, r = W+L-1)
    g = NCH - 1
    aN = (Vf[g][W + L - 1].astype(np.float64) + s64[N - 1]
          + _cf(W + L - 1) + df[g])
    m = aN.max()
    Z = m + np.log(np.exp(aN - m).sum())

    # per-position row constants
    i = np.arange(N)
    gf = i // L
    rfr = i - gf * L + np.where(gf > 0, W, 0)
    rev = N - 1 - i
    gb = rev // L
    rbr = rev - gb * L + np.where(gb > 0, W, 0)
    cf = _cf_vec(rfr) + df[gf]
    cb = _cf_vec(rbr) + db[gb]
    coef = ((rfr > 0).astype(np.float64) + (rbr > 0).astype(np.float64) - 1.0)
    rho = (cf + cb - Z).astype(np.float32)
    sp2 = (s64 * coef[:, None]).astype(np.float32)

    # gather lnV rows per position; fold the scores term into the fwd half
    LF = np.empty((N, C), np.float16)
    LBr = np.empty((N, C), np.float16)
    for g in range(NCH):
        warm = 0 if g == 0 else W
        LF[g * L:(g + 1) * L] = (
            Vf[g][warm:warm + L].astype(np.float32) + sp2[g * L:(g + 1) * L])
        LBr[g * L:(g + 1) * L] = Vb[g][warm:warm + L]
    LB = LBr[::-1]
    return LF, LB, rho


# ---------------------------------------------------------------- emulation
def emulate_scan_core(inmap):
    M16 = inmap["m16"]
    es = inmap["srows"].astype(np.float32)
    vst = np.zeros((P, R * NSC), np.float16)
    vst[:, 0:NSC] = np.log(
        es[:, 0:NSC] * np.float32(np.exp(MU - B0))).astype(np.float16)
    u = (es[:, 0:NSC] * np.float32(np.exp(MU - B0))).astype(np.float16)
    Mr = M16.astype(np.float32).reshape(P, CT, CT, P)   # [p, kt, jt, q]
    for r in range(1, R):
        U = u.astype(np.float32).reshape(P, CT, NS)
        ps = np.einsum('pkjq,pks->qjs', Mr, U, optimize=True)
        ps = ps.reshape(P, NSC)
        vst[:, r * NSC:(r + 1) * NSC] = np.log(ps).astype(np.float16)
        if r < R - 1:
            u = (ps * es[:, r * NSC:(r + 1) * NSC]).astype(np.float16)
    return {"vdump": vst}


# ---------------------------------------------------------------- main entry
def kernel(scores, T, simulate=False):
    import time
    global _scan_nc
    scores = np.ascontiguousarray(np.asarray(scores), dtype=np.float32)
    T = np.ascontiguousarray(np.asarray(T), dtype=np.float32)

    t0 = time.time()
    in1 = prep_scan_inputs(scores, T)
    TIMINGS["prep1"] = time.time() - t0

    t0 = time.time()
    if simulate:
        res1 = [emulate_scan_core(m) for m in in1]
    else:
        if _scan_nc is None:
            tb = time.time()
            _scan_nc = build_scan_nc()
            TIMINGS["build1"] = time.time() - tb
        res1 = run_bass_kernel_spmd(_scan_nc, in1, list(range(NCORE))).results
    TIMINGS["pass1"] = time.time() - t0

    t0 = time.time()
    LF, LB, rho = host_stitch(res1, scores)
    # final combine: alpha + beta - scores - Z, all constants folded in
    out = LF.astype(np.float32)
    out += LB.astype(np.float32)
    out += rho[:, None]
    TIMINGS["host"] = time.time() - t0
    return out
